# revision 1
# baseline (speedup 1.0000x reference)
"""ChannelGuidedAttn Trainium2 kernel.

Reference computation (per batch b):
    q  = x_pre[b]      reshaped (C, N),  C=512, N=H*W=4096
    kv = x_training[b] reshaped (C, N)
    energy[c,d] = <q[c,:], kv[d,:]>                      (C x C)
    att = softmax(max_d(energy) - energy, axis=-1)       == softmax(-energy)
        = exp(min_d(energy) - energy) / sum
    out = att @ kv  -> (C, H, W);  final softmax over W

Sharding: data-parallel over batch B=16 across 8 cores (2 batches/core).

Design: all transposes/casts are done on the HOST during sharding; the
device receives three pre-laid-out fp16 tensors per batch:
  - ktT[p, j, d] = kv[d, j*128+p]   (gemm1 moving operand, n on partitions)
  - qT [ct, p, j, c] = q[ct*128+c, j*128+p]  (gemm1 stationary, per c-tile)
  - kn [p, dt, n] = kv[dt*128+p, n] (gemm2 moving operand, natural layout)
Each layout is partition-major so every load DMA moves 8KB-contiguous
descriptor payloads (full bus efficiency, no 2x small-element penalty).

Device program per (batch, c-tile) task:
  g1(ct):  energy = qT(ct)^T @ ktT  (32 fp16 matmuls into one PSUM bank)
           min-reduce (DVE), att16 = exp(min - E) with sum accum (ACT),
           normalize att16 (DVE), attT via small DMA xbar transpose.
  g2(ct):  out = attT^T @ kn  (8 x 4 fp16 matmuls), exp (ACT, fp16 out),
           per-W-segment sums (DVE), reciprocal (DVE), normalize
           (alternating DVE/Pool), fp16 store per half-c-tile.

The 16 PE task slots and the single serialized DMA queue are explicitly
co-scheduled (see the schedule section): batch 0 runs its four gemm1s
back-to-back while loads stream in, batch 1's gemm1s interleave into batch
0's gemm2 phase, every load is positioned to complete just before its
consuming PE slot, and attT transposes (whose sem waits would head-of-line
block the SP DMA queue) are placed only where they are already ready.
PE runs at its arithmetic roofline with no mid-kernel stalls: total
~125us/core vs 109us of pure matmul (baseline was 308us).

gemm1 runs in plain fp16 (host-rounded inputs, fp32 PSUM accumulation):
measured absmax rel err 1.19e-2 against the f64 reference (gate 2e-2),
dominated by the fp16 rounding of q/kv feeding the huge (std ~64) energy
dot products. G1_MODE="f16q" adds a q-residual pass (err 7.1e-3) at
+27us PE if a larger margin is ever needed.
"""

import sys

import numpy as np

for _p in ("/opt/trn_rl_repo", "/root/.axon_site/_ro/trn_rl_repo"):
    if _p not in sys.path:
        sys.path.append(_p)

B = 16
N_CORES = 8
B_PER_CORE = B // N_CORES
C = 512
H = 64
W = 64
N = H * W
CT = C // 128  # 4 c-tiles / d-tiles
NJ = N // 128  # 32 n-chunks of 128

G1_MODE = "f16"  # "f16" (plain fp16) | "f16q" (q hi/lo split, 2-pass)


def build_program(g1_mode=None):
    from contextlib import ExitStack

    import concourse.mybir as mybir
    import concourse.tile as tile
    from concourse import bacc

    if g1_mode is None:
        g1_mode = G1_MODE
    assert g1_mode in ("f16", "f16q")
    q_split = g1_mode == "f16q"

    f32 = mybir.dt.float32
    f16 = mybir.dt.float16
    Alu = mybir.AluOpType
    Act = mybir.ActivationFunctionType
    Axis = mybir.AxisListType

    nc = bacc.Bacc()
    # Host-prepared layouts (see module docstring).
    ktT = nc.declare_dram_parameter("ktT", [B_PER_CORE, 128, NJ, C], f16, isOutput=False)
    qT = nc.declare_dram_parameter("qT", [B_PER_CORE, CT, 128, NJ, 128], f16, isOutput=False)
    if q_split:
        qlT = nc.declare_dram_parameter(
            "qlT", [B_PER_CORE, CT, 128, NJ, 128], f16, isOutput=False
        )
    kn = nc.declare_dram_parameter("kn", [B_PER_CORE, 128, CT, N], f16, isOutput=False)
    out = nc.declare_dram_parameter("out", [B_PER_CORE, C, N], f16, isOutput=True)

    with tile.TileContext(nc) as tc, ExitStack() as ctx:
        ktp = ctx.enter_context(tc.tile_pool(name="ktp", bufs=2))
        knp = ctx.enter_context(tc.tile_pool(name="knp", bufs=2))
        qtp = ctx.enter_context(tc.tile_pool(name="qtp", bufs=5 + 3 * q_split))
        attp = ctx.enter_context(tc.tile_pool(name="attp", bufs=2))
        ostp = ctx.enter_context(tc.tile_pool(name="ostp", bufs=5))
        small = ctx.enter_context(tc.tile_pool(name="small", bufs=4))
        ps_e = ctx.enter_context(tc.tile_pool(name="ps_e", bufs=3, space="PSUM"))
        ps_o = ctx.enter_context(tc.tile_pool(name="ps_o", bufs=4, space="PSUM"))

        # Per-batch SBUF tiles, created/rotated on demand.
        kt_sb = {}
        kn_sb = {}
        qt_sb = {}
        ql_sb = {}
        att_16 = {}
        att_T = {}

        def emit_ktT_chunks(b, chunks, nch=8):
            # default 8 chunks of 4 j's each (~1.6us apiece on the DMA queue)
            if b not in kt_sb:
                kt_sb[b] = ktp.tile([128, NJ, C], f16, tag="ktT", name=f"ktT_{b}")
            for g in chunks:
                js = slice(g * (NJ // nch), (g + 1) * (NJ // nch))
                nc.sync.dma_start(out=kt_sb[b][:, js, :], in_=ktT[b, :, js, :])

        def emit_kn_chunks(b, chunks):
            if b not in kn_sb:
                kn_sb[b] = knp.tile([128, CT, N], f16, tag="kn", name=f"kn_{b}")
            for dt in chunks:
                nc.sync.dma_start(out=kn_sb[b][:, dt, :], in_=kn[b, :, dt, :])

        def emit_qT_load(b, ct, halves=1):
            qt_sb[(b, ct)] = qtp.tile([128, NJ, 128], f16, tag="qT", name=f"qT_{b}_{ct}")
            for hh in range(halves):
                js = slice(hh * (NJ // halves), (hh + 1) * (NJ // halves))
                nc.sync.dma_start(out=qt_sb[(b, ct)][:, js, :], in_=qT[b, ct, :, js, :])
            if q_split:
                ql_sb[(b, ct)] = qtp.tile(
                    [128, NJ, 128], f16, tag="qlT", name=f"qlT_{b}_{ct}"
                )
                nc.sync.dma_start(out=ql_sb[(b, ct)], in_=qlT[b, ct])

        def emit_g1(b, ct):
            # energy for this c-tile, then attention row softmax + transpose
            e_ps = ps_e.tile([128, C], f32, tag="ps_e")
            qt = qt_sb.pop((b, ct))
            ql = ql_sb.pop((b, ct), None)
            kt = kt_sb[b]
            for j in range(NJ):
                last = j == NJ - 1
                nc.tensor.matmul(
                    e_ps, qt[:, j, :], kt[:, j, :],
                    start=(j == 0), stop=(last and not q_split),
                )
                if q_split:
                    nc.tensor.matmul(
                        e_ps, ql[:, j, :], kt[:, j, :], start=False, stop=last
                    )
            min_t = small.tile([128, 1], f32, tag="min")
            nc.vector.tensor_reduce(min_t, e_ps, axis=Axis.X, op=Alu.min)
            att16 = attp.tile([128, C], f16, tag="att16")
            den = small.tile([128, 1], f32, tag="den")
            nc.scalar.activation(
                out=att16, in_=e_ps, func=Act.Exp, bias=min_t, scale=-1.0,
                accum_out=den,
            )
            rden = small.tile([128, 1], f32, tag="rden")
            nc.vector.reciprocal(rden, den)
            nc.vector.tensor_scalar_mul(att16, att16, rden)
            att_16[(b, ct)] = att16

        def emit_attT(b, ct):
            # placed explicitly in the DMA queue: late enough that its sem
            # wait (att16 ready) never head-of-line-blocks loads behind it
            att_T[(b, ct)] = attp.tile(
                [128, CT, 128], f16, tag="attT", name=f"attT_{b}_{ct}"
            )
            nc.sync.dma_start_transpose(att_T[(b, ct)], att_16.pop((b, ct)))

        def emit_g2(b, ct, tail=False, last=False):
            # out rows for this c-tile: gemm2 + final softmax over W + store.
            # tail: the final two tasks — DVE is the scarce engine there, so
            # push most normalizes to Pool (DVE keeps nj 5,7 for low latency).
            attT = att_T.pop((b, ct))
            knb = kn_sb[b]
            for h in range(2):
                o16 = ostp.tile([128, 2048], f16, tag="ost")  # 4 nj chunks of 512

                def post(o_ps, k, nj, lo, hi, seg_tag, store):
                    # softmax over W + optional store for columns [lo, hi)
                    sl = slice(k * 512 + lo, k * 512 + hi)
                    o16v = o16[:, sl].rearrange("p (s w) -> p s w", w=W)
                    nc.scalar.activation(
                        out=o16v,
                        in_=o_ps[:, : hi - lo].rearrange("p (s w) -> p s w", w=W),
                        func=Act.Exp,
                    )
                    nseg = (hi - lo) // W
                    ssum = small.tile([128, nseg], f32, tag="ssum" + seg_tag)
                    nc.vector.tensor_reduce(ssum, o16v, axis=Axis.X, op=Alu.add)
                    rsum = small.tile([128, nseg], f32, tag="rsum" + seg_tag)
                    nc.vector.reciprocal(rsum, ssum)
                    if last and nj == 7:
                        # final chunk: split the normalize across DVE and
                        # Pool in parallel to shorten the last store's gate
                        hs = nseg // 2
                        for i, eng in ((0, nc.vector), (1, nc.gpsimd)):
                            ov = o16v[:, i * hs : (i + 1) * hs, :]
                            eng.tensor_tensor(
                                out=ov,
                                in0=ov,
                                in1=rsum[:, i * hs : (i + 1) * hs, None]
                                .to_broadcast(ov.shape),
                                op=Alu.mult,
                            )
                    else:
                        if tail:
                            on_pool = nj != 7
                        else:
                            on_pool = nj % 2 == 0
                        eng = nc.gpsimd if on_pool else nc.vector
                        eng.tensor_tensor(
                            out=o16v,
                            in0=o16v,
                            in1=rsum[:, :, None].to_broadcast(o16v.shape),
                            op=Alu.mult,
                        )
                    if store:
                        nc.sync.dma_start(
                            out=out[
                                b,
                                ct * 128 : (ct + 1) * 128,
                                nj * 512 + lo : nj * 512 + hi,
                            ],
                            in_=o16[:, sl],
                        )

                def mm(nj, lo, hi, o_ps):
                    for dt in range(CT):
                        nc.tensor.matmul(
                            o_ps[:, : hi - lo],
                            attT[:, dt, :],
                            knb[:, dt, nj * 512 + lo : nj * 512 + hi],
                            start=(dt == 0),
                            stop=(dt == CT - 1),
                        )

                for k in range(4):
                    nj = h * 4 + k
                    o_ps = ps_o.tile([128, 512], f32, tag="ps_o")
                    mm(nj, 0, 512, o_ps)
                    post(o_ps, k, nj, 0, 512, "", store=last)
                if not last:
                    nc.sync.dma_start(
                        out=out[
                            b, ct * 128 : (ct + 1) * 128, h * 2048 : (h + 1) * 2048
                        ],
                        in_=o16,
                    )

        # ---- explicit software-pipelined schedule (2 batches) ----
        # DMA queue and the in-order PE stream are co-scheduled: batch 0 runs
        # all four gemm1s first (its kn/attT can't be ready earlier anyway);
        # batch 1 prefetches are slotted so no PE task ever waits on a load.
        assert B_PER_CORE == 2
        # Hand-scheduled against the cost model: PE slot sequence is
        # g1(00..03), g2(00), g2(01), g1(10), g2(02), g1(11), g2(03),
        # g1(12), g2(10), g1(13), g2(11), g2(12), g2(13); every DMA is
        # placed so it completes just before its consuming PE slot and no
        # sem-waiting DMA ever blocks a load queued behind it.
        emit_qT_load(0, 0, halves=2)
        emit_ktT_chunks(0, range(0, 8), nch=16)
        emit_qT_load(0, 1, halves=2)
        emit_ktT_chunks(0, range(4, 8), nch=8)
        emit_qT_load(0, 2)
        emit_qT_load(0, 3)
        emit_kn_chunks(0, [0, 1, 2])
        emit_g1(0, 0)
        emit_attT(0, 0)
        emit_kn_chunks(0, [3])
        emit_g1(0, 1)
        emit_attT(0, 1)
        emit_g1(0, 2)
        emit_attT(0, 2)
        emit_qT_load(1, 0)
        emit_ktT_chunks(1, range(0, 2))
        emit_g1(0, 3)
        emit_attT(0, 3)
        emit_ktT_chunks(1, range(2, 8))
        emit_g2(0, 0)  # st00 x2
        emit_qT_load(1, 1)
        emit_kn_chunks(1, [0, 1])
        emit_g2(0, 1)  # st01 x2
        emit_g1(1, 0)
        emit_qT_load(1, 2)
        emit_kn_chunks(1, [2, 3])
        emit_attT(1, 0)
        emit_g2(0, 2)  # st02 x2
        emit_g1(1, 1)
        emit_qT_load(1, 3)
        emit_attT(1, 1)
        emit_g2(0, 3)  # st03 x2
        emit_g1(1, 2)
        emit_attT(1, 2)
        emit_g2(1, 0)  # st10 x2
        emit_g1(1, 3)
        emit_attT(1, 3)
        emit_g2(1, 1)  # st11 x2
        emit_g2(1, 2, tail=True)
        emit_g2(1, 3, tail=True, last=True)

    nc.finalize()
    return nc


def prepare_in_maps(x_training, x_pre, g1_mode=None):
    """Host-side shard + layout prep. Returns per-core input dicts."""
    if g1_mode is None:
        g1_mode = G1_MODE
    xt = np.asarray(x_training, dtype=np.float32).reshape(B, C, N)
    xp = np.asarray(x_pre, dtype=np.float32).reshape(B, C, N)
    xt16 = xt.astype(np.float16)
    xp16 = xp.astype(np.float16)

    # ktT[b, p, j, d] = kv[b, d, j*128+p]
    ktT = np.ascontiguousarray(
        xt16.reshape(B, C, NJ, 128).transpose(0, 3, 2, 1)
    )
    # qT[b, ct, p, j, c] = q[b, ct*128+c, j*128+p]
    qT = np.ascontiguousarray(
        xp16.reshape(B, CT, 128, NJ, 128).transpose(0, 1, 4, 3, 2)
    )
    # kn[b, p, dt, n] = kv[b, dt*128+p, n]
    knat = np.ascontiguousarray(xt16.reshape(B, CT, 128, N).transpose(0, 2, 1, 3))
    if g1_mode == "f16q":
        ql = (xp - xp16.astype(np.float32)).astype(np.float16)
        qlT = np.ascontiguousarray(
            ql.reshape(B, CT, 128, NJ, 128).transpose(0, 1, 4, 3, 2)
        )

    in_maps = []
    for i in range(N_CORES):
        sl = slice(i * B_PER_CORE, (i + 1) * B_PER_CORE)
        m = {"ktT": ktT[sl], "qT": qT[sl], "kn": knat[sl]}
        if g1_mode == "f16q":
            m["qlT"] = qlT[sl]
        in_maps.append(m)
    return in_maps


def kernel(x_training: np.ndarray, x_pre: np.ndarray) -> np.ndarray:
    from concourse.bass_utils import run_bass_kernel_spmd

    nc = build_program()
    in_maps = prepare_in_maps(x_training, x_pre)
    res = run_bass_kernel_spmd(nc, in_maps, list(range(N_CORES)))
    outs = [np.asarray(r["out"]) for r in res.results]
    return np.concatenate(outs, axis=0).reshape(B, C, H, W).astype(np.float32)



# revision 13
# speedup vs baseline: 1.1185x; 1.1185x over previous
"""ChannelGuidedAttn Trainium2 kernel (fp16 gemm1 + DoubleRow-fp8 gemm2).

Reference computation (per batch b):
    q  = x_pre[b]      reshaped (C, N),  C=512, N=H*W=4096
    kv = x_training[b] reshaped (C, N)
    energy[c,d] = <q[c,:], kv[d,:]>                      (C x C)
    att = softmax(max_d(energy) - energy, axis=-1)       == softmax(-energy)
    out = att @ kv  -> (C, H, W);  final softmax over W

Sharding: data-parallel over batch B=16 across 8 cores (2 batches/core).

Design (v2):
  gemm1 (energy) unchanged from baseline: fp16 qT (stationary) x fp16 ktT
  (moving), fp32 PSUM.  Softmax numerics, validated on HW:
    att8 = fp8(fp16(exp(min - E)))  -- UNnormalized (peak = 1.0, exact in
    fp8); den accumulated in fp32 by the ACT engine; rden = 1/den folded
    into gemm2's final exp as a per-partition scale.
  attT via PE transpose (4 x [128,128] fp16 tiles into PSUM) + ACT copy to
  fp8 SBUF -- off the DMA queue entirely.
  gemm2 in DoubleRow fp8 (cost model: 0.5 cyc/row, 2x128 contraction per
  MM): att8 stationary pairs over d-tiles, kv as host-prepared hi/lo fp8
  pair (kn[:, 0] = fp8(kv), kn[:, 1] = fp8(kv - fp8(kv))), 4 MMs per
  512-col chunk.  Measured end-to-end rel err 1.35e-2 (gate 2e-2).
  Final W-softmax: ACT exp(scale=rden) -> fp16, DVE segment sums + recip,
  normalize split DVE/Pool, fp16 store.

Engine budget per core (cost model): PE 54.6(g1) + 13.7(g2) + 1.7(T) us;
DMA 33.6MB at 360 B/ns = 93.3 us -> DMA-bound; ACT ~50 us, DVE ~55 us,
Pool ~20 us all under the DMA roof.  The schedule section orders the SP
DMA queue (the critical resource) and interleaves PE g1/g2 work so PE
never idles long enough to matter.
"""

import sys

import numpy as np

for _p in ("/opt/trn_rl_repo", "/root/.axon_site/_ro/trn_rl_repo"):
    if _p not in sys.path:
        sys.path.append(_p)

B = 16
N_CORES = 8
B_PER_CORE = B // N_CORES
C = 512
H = 64
W = 64
N = H * W
CT = C // 128  # 4 c-tiles / d-tiles
NJ = N // 128  # 32 n-chunks of 128


def build_program():
    from contextlib import ExitStack

    import concourse.mybir as mybir
    import concourse.tile as tile
    from concourse import bacc

    f32 = mybir.dt.float32
    f16 = mybir.dt.float16
    f8 = mybir.dt.float8e4
    Alu = mybir.AluOpType
    Act = mybir.ActivationFunctionType
    Axis = mybir.AxisListType
    DR = mybir.MatmulPerfMode.DoubleRow

    nc = bacc.Bacc()
    # Host-prepared layouts (see module docstring).
    ktT = nc.declare_dram_parameter("ktT", [B_PER_CORE, 128, NJ, C], f16, isOutput=False)
    qT = nc.declare_dram_parameter("qT", [B_PER_CORE, CT, 128, NJ, 128], f16, isOutput=False)
    kn = nc.declare_dram_parameter("kn", [B_PER_CORE, 128, 2, CT, N], f8, isOutput=False)
    ident = nc.declare_dram_parameter("ident", [128, 128], f16, isOutput=False)
    out = nc.declare_dram_parameter("out", [B_PER_CORE, C, N], f16, isOutput=True)

    with tile.TileContext(nc) as tc, ExitStack() as ctx:
        ktp = ctx.enter_context(tc.tile_pool(name="ktp", bufs=2))
        knp = ctx.enter_context(tc.tile_pool(name="knp", bufs=2))
        qtp = ctx.enter_context(tc.tile_pool(name="qtp", bufs=3))
        attp = ctx.enter_context(tc.tile_pool(name="attp", bufs=2))
        atp8 = ctx.enter_context(tc.tile_pool(name="atp8", bufs=3))
        ostp = ctx.enter_context(tc.tile_pool(name="ostp", bufs=4))
        small = ctx.enter_context(tc.tile_pool(name="small", bufs=6))
        ps_e = ctx.enter_context(tc.tile_pool(name="ps_e", bufs=2, space="PSUM"))
        ps_o = ctx.enter_context(tc.tile_pool(name="ps_o", bufs=2, space="PSUM"))
        ps_t = ctx.enter_context(tc.tile_pool(name="ps_t", bufs=2, space="PSUM"))
        sbi = ctx.enter_context(tc.tile_pool(name="sbi", bufs=1))

        id_sb = sbi.tile([128, 128], f16, tag="ident", name="ident")
        nc.sync.dma_start(out=id_sb, in_=ident[:, :])

        kt_sb = {}
        kn_sb = {}
        qt_sb = {}
        att_16 = {}
        att_ps = {}
        att_T8 = {}
        rden_t = {}
        o16_sb = {}

        def emit_ktT_chunks(b, ranges):
            # ranges: list of (j0, j1)
            if b not in kt_sb:
                kt_sb[b] = ktp.tile([128, NJ, C], f16, tag="ktT", name=f"ktT_{b}")
            for j0, j1 in ranges:
                js = slice(j0, j1)
                nc.sync.dma_start(out=kt_sb[b][:, js, :], in_=ktT[b, :, js, :])

        def emit_kn_plane(b, hl):
            if b not in kn_sb:
                kn_sb[b] = knp.tile([128, 2, CT, N], f8, tag="kn", name=f"kn_{b}")
            nc.sync.dma_start(out=kn_sb[b][:, hl, :, :], in_=kn[b, :, hl, :, :])

        def emit_qT_load(b, ct, halves=1):
            qt_sb[(b, ct)] = qtp.tile([128, NJ, 128], f16, tag="qT", name=f"qT_{b}_{ct}")
            for hh in range(halves):
                js = slice(hh * (NJ // halves), (hh + 1) * (NJ // halves))
                nc.sync.dma_start(out=qt_sb[(b, ct)][:, js, :], in_=qT[b, ct, :, js, :])

        def emit_g1_mms(b, ct, j0, j1):
            if j0 == 0:
                att_ps[(b, ct)] = ps_e.tile(
                    [128, C], f32, tag="ps_e", name=f"eps_{b}_{ct}"
                )
            e_ps = att_ps[(b, ct)]
            qt = qt_sb[(b, ct)]
            kt = kt_sb[b]
            for j in range(j0, j1):
                nc.tensor.matmul(
                    e_ps, qt[:, j, :], kt[:, j, :],
                    start=(j == 0), stop=(j == NJ - 1),
                )

        def emit_g1_post(b, ct):
            e_ps = att_ps[(b, ct)]
            qt_sb.pop((b, ct))
            min_t = small.tile([128, 1], f32, tag="min", name=f"min_{b}_{ct}")
            nc.vector.tensor_reduce(min_t, e_ps, axis=Axis.X, op=Alu.min)
            att16 = attp.tile([128, C], f16, tag="att16", name=f"att16_{b}_{ct}")
            den = small.tile([128, 1], f32, tag="den", name=f"den_{b}_{ct}")
            nc.scalar.activation(
                out=att16, in_=e_ps, func=Act.Exp, bias=min_t, scale=-1.0,
                accum_out=den,
            )
            att_16[(b, ct)] = att16
            att_ps.pop((b, ct))
            rden = small.tile([128, 1], f32, tag="rden", name=f"rden_{b}_{ct}")
            nc.vector.reciprocal(rden, den)
            rden_t[(b, ct)] = rden

        def emit_T(b, ct):
            att16 = att_16.pop((b, ct))
            tr_ps = ps_t.tile([128, CT, 128], f16, tag="ps_t", name=f"trps_{b}_{ct}")
            for t in range(CT):
                nc.tensor.transpose(
                    tr_ps[:, t, :], att16[:, t * 128 : (t + 1) * 128], id_sb
                )
            attT8 = atp8.tile([128, CT, 128], f8, tag="attT8", name=f"attT8_{b}_{ct}")
            nc.scalar.activation(out=attT8, in_=tr_ps, func=Act.Copy)
            att_T8[(b, ct)] = attT8

        def emit_g2_half(b, ct, h, tail=False, last=False):
            # 2 chunk-PAIRS per half: 8 DoubleRow MMs into a 2-bank PSUM
            # tile, one exp / segsum / recip / normalize per pair (halves
            # the per-instruction init overhead on ACT and DVE).
            attT8 = att_T8[(b, ct)]
            if h == 1:
                att_T8.pop((b, ct))
            knb = kn_sb[b]
            rden = rden_t[(b, ct)]
            if h == 0:
                o16_sb[(b, ct)] = ostp.tile(
                    [128, 4096], f16, tag="ost", name=f"o16_{b}_{ct}"
                )
            o16 = o16_sb[(b, ct)]
            for k2 in range(2):
                nj0 = h * 4 + 2 * k2
                o_ps = ps_o.tile(
                    [128, 1024], f32, tag="ps_o", name=f"ops_{b}_{ct}_{h}_{k2}"
                )
                for kk in range(2):
                    cols = slice((nj0 + kk) * 512, (nj0 + kk + 1) * 512)
                    ov = o_ps[:, kk * 512 : (kk + 1) * 512]
                    for i, (hl, p) in enumerate(((0, 0), (1, 0), (0, 1), (1, 1))):
                        nc.tensor.matmul(
                            ov,
                            attT8[:, 2 * p : 2 * p + 2, :],
                            knb[:, hl, 2 * p : 2 * p + 2, cols],
                            start=(i == 0), stop=(i == 3),
                            perf_mode=DR,
                        )
                o16v = o16[:, nj0 * 512 : (nj0 + 2) * 512].rearrange(
                    "p (s w) -> p s w", w=W
                )
                nc.scalar.activation(
                    out=o16v,
                    in_=o_ps.rearrange("p (s w) -> p s w", w=W),
                    func=Act.Exp, scale=rden,
                )
                ssum = small.tile(
                    [128, 16], f32, tag="ssum", name=f"ssum_{b}_{ct}_{h}_{k2}"
                )
                nc.vector.tensor_reduce(ssum, o16v, axis=Axis.X, op=Alu.add)
                rsum = small.tile(
                    [128, 16], f32, tag="rsum", name=f"rsum_{b}_{ct}_{h}_{k2}"
                )
                nc.vector.reciprocal(rsum, ssum)
                if last and k2 == 1:
                    # split the very last normalize across Pool/DVE to
                    # shorten the final store's gate
                    for i2, eng in ((0, nc.gpsimd), (1, nc.vector)):
                        ov2 = o16v[:, i2 * 8 : (i2 + 1) * 8, :]
                        eng.tensor_tensor(
                            out=ov2, in0=ov2,
                            in1=rsum[:, i2 * 8 : (i2 + 1) * 8, None]
                            .to_broadcast(ov2.shape),
                            op=Alu.mult,
                        )
                else:
                    nc.gpsimd.tensor_tensor(
                        out=o16v, in0=o16v,
                        in1=rsum[:, :, None].to_broadcast(o16v.shape),
                        op=Alu.mult,
                    )

        def emit_store(b, ct, split=1):
            # all stores sit at the tail of the SP queue (loads are done by
            # then) in readiness order, so no store ever blocks a load and
            # the DMA engines drain them back-to-back.
            o16 = o16_sb.pop((b, ct))
            for s in range(split):
                lo = s * (4096 // split)
                hi = (s + 1) * (4096 // split)
                nc.sync.dma_start(
                    out=out[b, ct * 128 : (ct + 1) * 128, lo:hi],
                    in_=o16[:, lo:hi],
                )

        # ---- explicit software-pipelined schedule (2 batches) ----
        # SP carries only loads (25.2MB, ~75us chain incl per-DMA overhead),
        # ordered so every g1 input lands just before its PE slot; stores
        # (8.4MB) issue from the Pool queue as each unit finishes and drain
        # the DMA engines behind the loads.  PE starts ~4us in (graduated
        # first ktT chunks) and must run nearly stall-free to keep the last
        # store off the critical path.
        assert B_PER_CORE == 2
        emit_qT_load(0, 0, halves=2)
        emit_ktT_chunks(0, [(0, 2), (2, 8)])
        emit_qT_load(0, 1)
        emit_ktT_chunks(0, [(8, 20), (20, 32)])
        emit_g1_mms(0, 0, 0, NJ)
        emit_g1_post(0, 0)
        emit_qT_load(0, 2)
        emit_kn_plane(0, 0)
        emit_g1_mms(0, 1, 0, NJ)
        emit_g1_post(0, 1)
        emit_T(0, 0)
        emit_qT_load(0, 3)
        emit_kn_plane(0, 1)
        emit_g1_mms(0, 2, 0, NJ)
        emit_g1_post(0, 2)
        emit_T(0, 1)
        emit_qT_load(1, 0)
        emit_g1_mms(0, 3, 0, NJ)
        emit_g1_post(0, 3)
        emit_T(0, 2)
        emit_ktT_chunks(1, [(0, 16)])
        emit_g2_half(0, 0, 0)
        emit_T(0, 3)
        emit_g2_half(0, 0, 1)
        emit_ktT_chunks(1, [(16, 32)])
        emit_g2_half(0, 1, 0)
        emit_qT_load(1, 1)
        emit_g2_half(0, 1, 1)
        emit_g1_mms(1, 0, 0, 16)
        emit_qT_load(1, 2)
        emit_g2_half(0, 2, 0)
        emit_g1_mms(1, 0, 16, 32)
        emit_g1_post(1, 0)
        emit_kn_plane(1, 0)
        emit_g2_half(0, 2, 1)
        emit_g1_mms(1, 1, 0, 16)
        emit_kn_plane(1, 1)
        emit_T(1, 0)
        emit_g2_half(0, 3, 0)
        emit_g1_mms(1, 1, 16, 32)
        emit_g1_post(1, 1)
        emit_qT_load(1, 3)
        emit_T(1, 1)
        emit_g1_mms(1, 2, 0, 32)
        emit_g1_post(1, 2)
        emit_T(1, 2)
        emit_g2_half(0, 3, 1)
        emit_g2_half(1, 0, 0, tail=True)
        emit_g2_half(1, 0, 1, tail=True)
        emit_store(0, 0)
        emit_store(0, 1)
        emit_g2_half(1, 1, 0, tail=True)
        emit_g2_half(1, 1, 1, tail=True)
        emit_store(0, 2)
        emit_g2_half(1, 2, 0, tail=True)
        emit_g2_half(1, 2, 1, tail=True)
        emit_store(0, 3)
        emit_g1_mms(1, 3, 0, 32)
        emit_g1_post(1, 3)
        emit_T(1, 3)
        emit_store(1, 0)
        emit_g2_half(1, 3, 0, tail=True)
        emit_store(1, 1)
        emit_g2_half(1, 3, 1, tail=True, last=True)
        emit_store(1, 2)
        emit_store(1, 3, split=2)

    nc.finalize()
    return nc


def prepare_in_maps(x_training, x_pre):
    """Host-side shard + layout prep. Returns per-core input dicts."""
    import ml_dtypes

    np8 = ml_dtypes.float8_e4m3

    xt = np.asarray(x_training, dtype=np.float32).reshape(B, C, N)
    xp = np.asarray(x_pre, dtype=np.float32).reshape(B, C, N)
    xt16 = xt.astype(np.float16)
    xp16 = xp.astype(np.float16)

    # ktT[b, p, j, d] = kv[b, d, j*128+p]
    ktT = np.ascontiguousarray(xt16.reshape(B, C, NJ, 128).transpose(0, 3, 2, 1))
    # qT[b, ct, p, j, c] = q[b, ct*128+c, j*128+p]
    qT = np.ascontiguousarray(
        xp16.reshape(B, CT, 128, NJ, 128).transpose(0, 1, 4, 3, 2)
    )
    # kn[b, p, hl, dt, n]: hi/lo fp8 split of kv, d-major
    kh = xt.astype(np8)
    kl = (xt - kh.astype(np.float32)).astype(np8)
    knat = np.ascontiguousarray(
        np.stack([kh, kl], axis=1)  # B, 2, C, N
        .reshape(B, 2, CT, 128, N)
        .transpose(0, 3, 1, 2, 4)  # B, 128, 2, CT, N
    )
    identm = np.eye(128, dtype=np.float16)

    in_maps = []
    for i in range(N_CORES):
        sl = slice(i * B_PER_CORE, (i + 1) * B_PER_CORE)
        in_maps.append(
            {"ktT": ktT[sl], "qT": qT[sl], "kn": knat[sl], "ident": identm}
        )
    return in_maps


def kernel(x_training: np.ndarray, x_pre: np.ndarray) -> np.ndarray:
    from concourse.bass_utils import run_bass_kernel_spmd

    nc = build_program()
    in_maps = prepare_in_maps(x_training, x_pre)
    res = run_bass_kernel_spmd(nc, in_maps, list(range(N_CORES)))
    outs = [np.asarray(r["out"]) for r in res.results]
    return np.concatenate(outs, axis=0).reshape(B, C, H, W).astype(np.float32)


# revision 17
# speedup vs baseline: 1.1274x; 1.0079x over previous
"""ChannelGuidedAttn Trainium2 kernel (fp16 gemm1 + DoubleRow-fp8 gemm2).

Reference computation (per batch b):
    q  = x_pre[b]      reshaped (C, N),  C=512, N=H*W=4096
    kv = x_training[b] reshaped (C, N)
    energy[c,d] = <q[c,:], kv[d,:]>                      (C x C)
    att = softmax(max_d(energy) - energy, axis=-1)       == softmax(-energy)
    out = att @ kv  -> (C, H, W);  final softmax over W

Sharding: data-parallel over batch B=16 across 8 cores (2 batches/core).

Design (validated on HW: rel err 1.355e-2 vs 2e-2 gate, 110930 ns modeled
vs 124073 baseline):
  gemm1 (energy): fp16 qT (stationary) x fp16 ktT (moving), fp32 PSUM --
  energy precision is the error budget's dominant term and every fp8
  variant fails (hi/lo fp8 operands carry ~2^-8.6 vs fp16's 2^-11; the
  std-64 logits amplify).
  Softmax: att8 = fp8(fp16(exp(min - E))), UNnormalized so the peak is
  exactly 1.0 in fp8; den accumulated in fp32 by the ACT engine
  (accum_out sums pre-cast values); rden = 1/den folded into gemm2's
  final exp as a per-partition fp32 scale AP -- normalization costs zero
  engine time.
  attT: 4x PE transpose (fp16 -> PSUM) + ACT copy -> fp8 SBUF; entirely
  off the DMA queue.
  gemm2: DoubleRow fp8 (0.5 cyc/row, 2x128 contraction/MM).  Stationary
  att8 pairs over d-tiles; moving kv as host-split hi/lo fp8 planes
  (kn[:,0]=fp8(kv), kn[:,1]=fp8(kv-fp8(kv))) -- kv error hits the final
  logits directly, so single-fp8 kv fails (9.8e-2) while hi/lo passes.
  4 MMs per 512-col chunk, chunks processed in PAIRS into a 2-bank PSUM
  tile so one exp/segsum/recip/normalize covers 1024 cols (halves ACT/DVE
  per-instruction init overhead).
  Final W-softmax: ACT exp(scale=rden) -> fp16, DVE segment sums +
  reciprocal, normalize on Pool (DVE+Pool split on the last unit to
  shorten the final store's gate), fp16 store.

Timeline (cost model): the single DMA-engines resource is the floor --
25.2MB of loads (~76.5us incl ~250ns/DMA chain overhead) followed by
8.4MB of stores (~26us).  The SP queue carries loads in an order that
feeds PE just in time (graduated first ktT chunks for an early PE start;
qT(1,2)/qT(1,3) ahead of the kn(1) planes so the last g1 units are never
load-starved); all stores sit behind the loads in readiness order.  PE
(85.5us busy) interleaves per-unit: g1 j-ranges fill gemm2's ACT-paced
gaps, the three earlier batch-1 g2 units run before g1(1,3) so their
DVE/Pool posts drain while PE does the final g1, leaving only the last
unit's post chain exposed at the tail.
"""

import sys

import numpy as np

for _p in ("/opt/trn_rl_repo", "/root/.axon_site/_ro/trn_rl_repo"):
    if _p not in sys.path:
        sys.path.append(_p)

B = 16
N_CORES = 8
B_PER_CORE = B // N_CORES
C = 512
H = 64
W = 64
N = H * W
CT = C // 128  # 4 c-tiles / d-tiles
NJ = N // 128  # 32 n-chunks of 128


def build_program():
    from contextlib import ExitStack

    import concourse.mybir as mybir
    import concourse.tile as tile
    from concourse import bacc

    f32 = mybir.dt.float32
    f16 = mybir.dt.float16
    f8 = mybir.dt.float8e4
    Alu = mybir.AluOpType
    Act = mybir.ActivationFunctionType
    Axis = mybir.AxisListType
    DR = mybir.MatmulPerfMode.DoubleRow

    nc = bacc.Bacc()
    # Host-prepared layouts (see module docstring).
    ktT = nc.declare_dram_parameter("ktT", [B_PER_CORE, 128, NJ, C], f16, isOutput=False)
    qT = nc.declare_dram_parameter("qT", [B_PER_CORE, CT, 128, NJ, 128], f16, isOutput=False)
    kn = nc.declare_dram_parameter("kn", [B_PER_CORE, 128, 2, CT, N], f8, isOutput=False)
    ident = nc.declare_dram_parameter("ident", [128, 128], f16, isOutput=False)
    out = nc.declare_dram_parameter("out", [B_PER_CORE, C, N], f16, isOutput=True)

    with tile.TileContext(nc) as tc, ExitStack() as ctx:
        ktp = ctx.enter_context(tc.tile_pool(name="ktp", bufs=2))
        knp = ctx.enter_context(tc.tile_pool(name="knp", bufs=2))
        qtp = ctx.enter_context(tc.tile_pool(name="qtp", bufs=3))
        attp = ctx.enter_context(tc.tile_pool(name="attp", bufs=2))
        atp8 = ctx.enter_context(tc.tile_pool(name="atp8", bufs=3))
        ostp = ctx.enter_context(tc.tile_pool(name="ostp", bufs=4))
        small = ctx.enter_context(tc.tile_pool(name="small", bufs=6))
        ps_e = ctx.enter_context(tc.tile_pool(name="ps_e", bufs=2, space="PSUM"))
        ps_o = ctx.enter_context(tc.tile_pool(name="ps_o", bufs=2, space="PSUM"))
        ps_t = ctx.enter_context(tc.tile_pool(name="ps_t", bufs=2, space="PSUM"))
        sbi = ctx.enter_context(tc.tile_pool(name="sbi", bufs=1))

        id_sb = sbi.tile([128, 128], f16, tag="ident", name="ident")

        kt_sb = {}
        kn_sb = {}
        qt_sb = {}
        att_16 = {}
        att_ps = {}
        att_T8 = {}
        rden_t = {}
        o16_sb = {}

        def emit_ktT_chunks(b, ranges):
            # ranges: list of (j0, j1)
            if b not in kt_sb:
                kt_sb[b] = ktp.tile([128, NJ, C], f16, tag="ktT", name=f"ktT_{b}")
            for j0, j1 in ranges:
                js = slice(j0, j1)
                nc.sync.dma_start(out=kt_sb[b][:, js, :], in_=ktT[b, :, js, :])

        def emit_kn_plane(b, hl):
            if b not in kn_sb:
                kn_sb[b] = knp.tile([128, 2, CT, N], f8, tag="kn", name=f"kn_{b}")
            nc.sync.dma_start(out=kn_sb[b][:, hl, :, :], in_=kn[b, :, hl, :, :])

        def emit_qT_load(b, ct, halves=1):
            qt_sb[(b, ct)] = qtp.tile([128, NJ, 128], f16, tag="qT", name=f"qT_{b}_{ct}")
            for hh in range(halves):
                js = slice(hh * (NJ // halves), (hh + 1) * (NJ // halves))
                nc.sync.dma_start(out=qt_sb[(b, ct)][:, js, :], in_=qT[b, ct, :, js, :])

        def emit_g1_mms(b, ct, j0, j1):
            if j0 == 0:
                att_ps[(b, ct)] = ps_e.tile(
                    [128, C], f32, tag="ps_e", name=f"eps_{b}_{ct}"
                )
            e_ps = att_ps[(b, ct)]
            qt = qt_sb[(b, ct)]
            kt = kt_sb[b]
            for j in range(j0, j1):
                nc.tensor.matmul(
                    e_ps, qt[:, j, :], kt[:, j, :],
                    start=(j == 0), stop=(j == NJ - 1),
                )

        def emit_g1_post(b, ct):
            e_ps = att_ps[(b, ct)]
            qt_sb.pop((b, ct))
            min_t = small.tile([128, 1], f32, tag="min", name=f"min_{b}_{ct}")
            nc.vector.tensor_reduce(min_t, e_ps, axis=Axis.X, op=Alu.min)
            att16 = attp.tile([128, C], f16, tag="att16", name=f"att16_{b}_{ct}")
            den = small.tile([128, 1], f32, tag="den", name=f"den_{b}_{ct}")
            nc.scalar.activation(
                out=att16, in_=e_ps, func=Act.Exp, bias=min_t, scale=-1.0,
                accum_out=den,
            )
            att_16[(b, ct)] = att16
            att_ps.pop((b, ct))
            rden = small.tile([128, 1], f32, tag="rden", name=f"rden_{b}_{ct}")
            nc.vector.reciprocal(rden, den)
            rden_t[(b, ct)] = rden

        def emit_T(b, ct):
            att16 = att_16.pop((b, ct))
            tr_ps = ps_t.tile([128, CT, 128], f16, tag="ps_t", name=f"trps_{b}_{ct}")
            for t in range(CT):
                nc.tensor.transpose(
                    tr_ps[:, t, :], att16[:, t * 128 : (t + 1) * 128], id_sb
                )
            attT8 = atp8.tile([128, CT, 128], f8, tag="attT8", name=f"attT8_{b}_{ct}")
            nc.scalar.activation(out=attT8, in_=tr_ps, func=Act.Copy)
            att_T8[(b, ct)] = attT8

        def emit_g2_half(b, ct, h, tail=False, last=False):
            # 2 chunk-PAIRS per half: 8 DoubleRow MMs into a 2-bank PSUM
            # tile, one exp / segsum / recip / normalize per pair (halves
            # the per-instruction init overhead on ACT and DVE).
            attT8 = att_T8[(b, ct)]
            if h == 1:
                att_T8.pop((b, ct))
            knb = kn_sb[b]
            rden = rden_t[(b, ct)]
            if h == 0:
                o16_sb[(b, ct)] = ostp.tile(
                    [128, 4096], f16, tag="ost", name=f"o16_{b}_{ct}"
                )
            o16 = o16_sb[(b, ct)]
            for k2 in range(2):
                nj0 = h * 4 + 2 * k2
                o_ps = ps_o.tile(
                    [128, 1024], f32, tag="ps_o", name=f"ops_{b}_{ct}_{h}_{k2}"
                )
                for kk in range(2):
                    cols = slice((nj0 + kk) * 512, (nj0 + kk + 1) * 512)
                    ov = o_ps[:, kk * 512 : (kk + 1) * 512]
                    for i, (hl, p) in enumerate(((0, 0), (1, 0), (0, 1), (1, 1))):
                        nc.tensor.matmul(
                            ov,
                            attT8[:, 2 * p : 2 * p + 2, :],
                            knb[:, hl, 2 * p : 2 * p + 2, cols],
                            start=(i == 0), stop=(i == 3),
                            perf_mode=DR,
                        )
                o16v = o16[:, nj0 * 512 : (nj0 + 2) * 512].rearrange(
                    "p (s w) -> p s w", w=W
                )
                nc.scalar.activation(
                    out=o16v,
                    in_=o_ps.rearrange("p (s w) -> p s w", w=W),
                    func=Act.Exp, scale=rden,
                )
                ssum = small.tile(
                    [128, 16], f32, tag="ssum", name=f"ssum_{b}_{ct}_{h}_{k2}"
                )
                nc.vector.tensor_reduce(ssum, o16v, axis=Axis.X, op=Alu.add)
                rsum = small.tile(
                    [128, 16], f32, tag="rsum", name=f"rsum_{b}_{ct}_{h}_{k2}"
                )
                nc.vector.reciprocal(rsum, ssum)
                if last and k2 == 1:
                    # split the very last normalize across Pool/DVE to
                    # shorten the final store's gate
                    for i2, eng in ((0, nc.gpsimd), (1, nc.vector)):
                        ov2 = o16v[:, i2 * 8 : (i2 + 1) * 8, :]
                        eng.tensor_tensor(
                            out=ov2, in0=ov2,
                            in1=rsum[:, i2 * 8 : (i2 + 1) * 8, None]
                            .to_broadcast(ov2.shape),
                            op=Alu.mult,
                        )
                else:
                    nc.gpsimd.tensor_tensor(
                        out=o16v, in0=o16v,
                        in1=rsum[:, :, None].to_broadcast(o16v.shape),
                        op=Alu.mult,
                    )

        def emit_store(b, ct, split=1):
            # all stores sit at the tail of the SP queue (loads are done by
            # then) in readiness order, so no store ever blocks a load and
            # the DMA engines drain them back-to-back.
            o16 = o16_sb.pop((b, ct))
            for s in range(split):
                lo = s * (4096 // split)
                hi = (s + 1) * (4096 // split)
                nc.sync.dma_start(
                    out=out[b, ct * 128 : (ct + 1) * 128, lo:hi],
                    in_=o16[:, lo:hi],
                )

        # ---- explicit software-pipelined schedule (2 batches) ----
        # SP carries only loads (25.2MB, ~75us chain incl per-DMA overhead),
        # ordered so every g1 input lands just before its PE slot; stores
        # (8.4MB) issue from the Pool queue as each unit finishes and drain
        # the DMA engines behind the loads.  PE starts ~4us in (graduated
        # first ktT chunks) and must run nearly stall-free to keep the last
        # store off the critical path.
        assert B_PER_CORE == 2
        emit_qT_load(0, 0, halves=2)
        emit_ktT_chunks(0, [(0, 2), (2, 8)])
        emit_qT_load(0, 1)
        emit_ktT_chunks(0, [(8, 20)])
        nc.sync.dma_start(out=id_sb, in_=ident[:, :])
        emit_ktT_chunks(0, [(20, 32)])
        emit_g1_mms(0, 0, 0, NJ)
        emit_g1_post(0, 0)
        emit_qT_load(0, 2)
        emit_kn_plane(0, 0)
        emit_g1_mms(0, 1, 0, NJ)
        emit_g1_post(0, 1)
        emit_T(0, 0)
        emit_qT_load(0, 3)
        emit_kn_plane(0, 1)
        emit_g1_mms(0, 2, 0, NJ)
        emit_g1_post(0, 2)
        emit_T(0, 1)
        emit_qT_load(1, 0)
        emit_g1_mms(0, 3, 0, NJ)
        emit_g1_post(0, 3)
        emit_T(0, 2)
        emit_ktT_chunks(1, [(0, 16)])
        emit_g2_half(0, 0, 0)
        emit_T(0, 3)
        emit_g2_half(0, 0, 1)
        emit_ktT_chunks(1, [(16, 32)])
        emit_g2_half(0, 1, 0)
        emit_qT_load(1, 1)
        emit_g2_half(0, 1, 1)
        emit_g1_mms(1, 0, 0, 16)
        emit_qT_load(1, 2)
        emit_g2_half(0, 2, 0)
        emit_g1_mms(1, 0, 16, 32)
        emit_g1_post(1, 0)
        emit_kn_plane(1, 0)
        emit_g2_half(0, 2, 1)
        emit_g1_mms(1, 1, 0, 16)
        emit_kn_plane(1, 1)
        emit_T(1, 0)
        emit_g2_half(0, 3, 0)
        emit_g1_mms(1, 1, 16, 32)
        emit_g1_post(1, 1)
        emit_qT_load(1, 3)
        emit_T(1, 1)
        emit_g1_mms(1, 2, 0, 32)
        emit_g1_post(1, 2)
        emit_T(1, 2)
        emit_g2_half(0, 3, 1)
        emit_g2_half(1, 0, 0, tail=True)
        emit_g2_half(1, 0, 1, tail=True)
        emit_store(0, 0)
        emit_store(0, 1)
        emit_g2_half(1, 1, 0, tail=True)
        emit_g2_half(1, 1, 1, tail=True)
        emit_store(0, 2)
        emit_g2_half(1, 2, 0, tail=True)
        emit_g2_half(1, 2, 1, tail=True)
        emit_store(0, 3)
        emit_g1_mms(1, 3, 0, 32)
        emit_g1_post(1, 3)
        emit_T(1, 3)
        emit_store(1, 0)
        emit_g2_half(1, 3, 0, tail=True)
        emit_store(1, 1)
        emit_g2_half(1, 3, 1, tail=True, last=True)
        emit_store(1, 2)
        emit_store(1, 3, split=4)

    nc.finalize()
    return nc


def prepare_in_maps(x_training, x_pre):
    """Host-side shard + layout prep. Returns per-core input dicts."""
    import ml_dtypes

    np8 = ml_dtypes.float8_e4m3

    xt = np.asarray(x_training, dtype=np.float32).reshape(B, C, N)
    xp = np.asarray(x_pre, dtype=np.float32).reshape(B, C, N)
    xt16 = xt.astype(np.float16)
    xp16 = xp.astype(np.float16)

    # ktT[b, p, j, d] = kv[b, d, j*128+p]
    ktT = np.ascontiguousarray(xt16.reshape(B, C, NJ, 128).transpose(0, 3, 2, 1))
    # qT[b, ct, p, j, c] = q[b, ct*128+c, j*128+p]
    qT = np.ascontiguousarray(
        xp16.reshape(B, CT, 128, NJ, 128).transpose(0, 1, 4, 3, 2)
    )
    # kn[b, p, hl, dt, n]: hi/lo fp8 split of kv, d-major
    kh = xt.astype(np8)
    kl = (xt - kh.astype(np.float32)).astype(np8)
    knat = np.ascontiguousarray(
        np.stack([kh, kl], axis=1)  # B, 2, C, N
        .reshape(B, 2, CT, 128, N)
        .transpose(0, 3, 1, 2, 4)  # B, 128, 2, CT, N
    )
    identm = np.eye(128, dtype=np.float16)

    in_maps = []
    for i in range(N_CORES):
        sl = slice(i * B_PER_CORE, (i + 1) * B_PER_CORE)
        in_maps.append(
            {"ktT": ktT[sl], "qT": qT[sl], "kn": knat[sl], "ident": identm}
        )
    return in_maps


def kernel(x_training: np.ndarray, x_pre: np.ndarray) -> np.ndarray:
    from concourse.bass_utils import run_bass_kernel_spmd

    nc = build_program()
    in_maps = prepare_in_maps(x_training, x_pre)
    res = run_bass_kernel_spmd(nc, in_maps, list(range(N_CORES)))
    outs = [np.asarray(r["out"]) for r in res.results]
    return np.concatenate(outs, axis=0).reshape(B, C, H, W).astype(np.float32)
